# revision 1
# baseline (speedup 1.0000x reference)
"""Trainium2 Bass kernel for nn_ArgmaxPositions (argmax-position relevance scatter).

Reference computation (per (i,j,c) of a [39,39,64] grid):
  k* = argmax_{k in 256} patch(i,j)[k] * w[k,c]   (k = (px,py,pc) = px*32+py*4+pc)
  out[4i+px*, 4j+py*, pc*] += rel[i,j,c]
Output: [1,160,160,4] float32.

Distribution (8 NeuronCores, SPMD):
  - Shard Cout=64 -> 8 channels per core. Each core computes argmax+scatter for
    its channels over the full 39x39 grid into a private relevance map; a
    ReduceScatter(add) sums the maps and leaves each core a 20-gx-row slice.
  - Per core: 13 tiles of (3 i-rows x 39 j) = 117 partitions.
    DVE: prod = patch*w (broadcast over c) -> reduce_max over k -> is_equal
    (one-hot, bf16, written transposed [117,256,8]); Pool: one-hot *= rel;
    DVE: reduce_add over c -> P[117,256].  The Pool stage is hidden by
    double-buffering the one-hot and running DVE's reduce one tile behind;
    patches are triple-buffered and prefetched two tiles ahead.
  - col2im: with stride 4 / filter 8, parity groups (i%2,j%2) tile the output
    plane disjointly -> per-tile strided DMAs scatter P straight from SBUF into
    4 DRAM canvases (collision-free), overlapped with compute; canvases are
    summed with vector adds and ReduceScatter'ed at the end.
"""

import numpy as np

H_IN, W_IN, C_IN = 160, 160, 4
H_OUT, W_OUT, C_OUT = 39, 39, 64
F, S = 8, 4
N_CORES = 8
C_SH = C_OUT // N_CORES          # 8 output channels per core
K = F * F * C_IN                 # 256 patch positions
TILE_I = 3
N_TILES = H_OUT // TILE_I        # 13
NP = TILE_I * W_OUT              # 117 partitions per tile
GX_SH = H_IN // N_CORES          # 20 output rows per core
OUT_FLAT = H_IN * W_IN * C_IN    # 102400
RS_SH = OUT_FLAT // N_CORES      # 12800
FILLS_PER_TILE = 2 * TILE_I      # (b parity) x (i rows)


def _build_nc(with_tail=True, with_compute=True):
    from contextlib import ExitStack

    from concourse import bass
    import concourse.mybir as mybir

    f32 = mybir.dt.float32
    bf16 = mybir.dt.bfloat16
    AP = bass.AP
    Alu = mybir.AluOpType
    Axis = mybir.AxisListType

    nc = bass.Bass(target_bir_lowering=False, debug=True)

    patches_ext = nc.declare_dram_parameter(
        "patches", [N_TILES, NP, K], f32, isOutput=False
    )
    w_ext = nc.declare_dram_parameter("w", [C_SH, K], f32, isOutput=False)
    rel_ext = nc.declare_dram_parameter("rel", [NP, N_TILES, C_SH], bf16, isOutput=False)
    out_ext = nc.declare_dram_parameter("out", [GX_SH, W_IN, C_IN], f32, isOutput=True)

    canv = nc.dram_tensor("canv", [4, H_IN, W_IN, C_IN], bf16)
    ar_in = nc.dram_tensor("ar_in", [OUT_FLAT], f32)
    rs_out = nc.dram_tensor("rs_out", [RS_SH], f32)

    # DRAM element strides
    xs_r, xs_c = W_IN * C_IN, C_IN          # x[row, col, pc]

    with ExitStack() as ctx:
        block = ctx.enter_context(nc.Block())
        sem = lambda name: ctx.enter_context(nc.semaphore(name))
        zw_sem = sem("zw_sem")
        patch_semA = sem("patch_semA")
        patch_semB = sem("patch_semB")
        patch_semC = sem("patch_semC")
        fill_sem0 = sem("fill_sem0")
        fill_sem1 = sem("fill_sem1")
        zc_sem = sem("zc_sem")        # canvas zero DMAs
        rb_sem = sem("rb_sem")
        zwb_sem = sem("zwb_sem")   # Pool w half
        fsc_sem = sem("fsc_sem")   # scalar-issued tile-12 fills
        ar_sem = sem("ar_sem")
        z_sem = sem("z_sem")          # zero-tile memset done
        ve_sem = sem("ve_sem")        # DVE eq(t) milestones
        vr_sem = sem("vr_sem")        # DVE reduce(t) milestones
        vt_sem = sem("vt_sem")        # DVE intra-tile chain (mult/max/adds)
        p_sem = sem("p_sem")          # Pool mult milestones
        cc_sem = sem("cc_sem")
        va_sem = sem("va_sem")        # final acc sum done

        rbb_sem0 = sem("rbb_sem0")    # band readback DMAs (ping/pong)
        rbb_sem1 = sem("rbb_sem1")
        pb_sem = sem("pb_sem")        # Pool intra-band add chain
        pba_sem = sem("pba_sem")      # Pool band-acc done milestones
        arb_sem0 = sem("arb_sem0")    # ar_in band DMAs (ping/pong)
        arb_sem1 = sem("arb_sem1")
        vo_sem = sem("vo_sem")        # out cast done
        vrb_sem = sem("vrb_sem")      # last-tile reduce, upper k half
        zwc_sem = sem("zwc_sem")      # scalar-fetched upper w quarter
        pe_sem = sem("pe_sem")        # Pool eq-half milestones

        sb = lambda *a: ctx.enter_context(nc.sbuf_tensor(*a))
        w_rep = sb("w_rep", [NP, C_SH, K], f32)
        patch_sb = sb("patch_sb", [NP, 3, K], f32)
        prod = sb("prod", [NP, 2, C_SH, K], f32)
        mvals = sb("mvals", [NP, C_SH], f32)
        onehot = sb("onehot", [NP, 2, K, C_SH], bf16)
        Pbuf = sb("Pbuf", [NP, 2, K], bf16)
        rel_bf = sb("rel_bf", [NP, N_TILES, C_SH], bf16)
        zero_t = sb("zero_t", [128, 800], bf16)
        bigrb = sb("bigrb", [128, 4, 800], bf16)
        acc_bf = sb("acc_bf", [128, 800], f32)
        rs_sb = sb("rs_sb", [128, 100], bf16)
        out_sb = sb("out_sb", [128, 100], f32)

        patch_sems = [patch_semA, patch_semB, patch_semC]
        fill_sems = [fill_sem0, fill_sem1]

        def n_fill(t):  # same-parity fill groups through tile t
            return t // 2 + 1

        # ---------------- sync engine: all DMA traffic ----------------
        # scalar engine: separate DMA queue for the big w_rep broadcast, so
        # patch DMAs (sync queue) aren't stuck behind 936KB
        @block.scalar
        def _(scalar: bass.BassScalarEngine):
            # c<2 (DVE's prod half) first, then rel; Pool fetches c>=2
            scalar.dma_start(
                out=w_rep[:, 0:2, :],
                in_=AP(w_ext, 0, [[0, NP], [K, 2], [1, K]]),
            ).then_inc(zw_sem, 16)
            scalar.dma_start(
                out=rel_bf[:, :, :],
                in_=rel_ext[:, :, :],
            ).then_inc(rb_sem, 16)

            if with_tail:
                # low-row (gx<140) readback of canvases 2:4 on this queue;
                # rows >=144 are only touched by tile 12, rows<140 final
                # after fills(11)/fills(10)
                if with_compute:
                    scalar.wait_ge(
                        fill_sems[1], 16 * FILLS_PER_TILE * n_fill(N_TILES - 2)
                    )
                    scalar.wait_ge(
                        fill_sems[0], 16 * FILLS_PER_TILE * n_fill(N_TILES - 3)
                    )
                else:
                    scalar.wait_ge(zc_sem, 16 * 4)
                scalar.dma_start(
                    out=bigrb[0:112, 2:4, :],
                    in_=AP(canv, 2 * OUT_FLAT, [[800, 112], [OUT_FLAT, 2], [1, 800]]),
                ).then_inc(rbb_sem1, 16)
                if with_compute:
                    # tile 12's il=2 fills on this queue, parallel to sync's
                    t12 = N_TILES - 1
                    i12 = TILE_I * t12 + 2
                    a12 = i12 % 2
                    scalar.wait_ge(vr_sem, N_TILES)
                    for b in range(2):
                        nj = (W_OUT - b + 1) // 2
                        p0 = 2 * W_OUT + (0 if b == 0 else (W_OUT + 1) // 2)
                        scalar.dma_start(
                            out=AP(
                                canv,
                                (2 * a12 + b) * OUT_FLAT
                                + 4 * i12 * xs_r
                                + 4 * b * xs_c,
                                [[8 * xs_c, nj], [xs_r, F], [1, F * C_IN]],
                            ),
                            in_=Pbuf[p0 : p0 + nj, t12 % 2, :],
                        ).then_inc(fsc_sem, 16)

        @block.sync
        def _(sync: bass.BassEngine):

            def issue_patch(t):
                # patches are pre-gathered (im2col) host-side: one contiguous DMA
                sync.dma_start(
                    out=patch_sb[:, t % 3, :],
                    in_=AP(patches_ext, t * NP * K, [[K, NP], [1, K]]),
                ).then_inc(patch_sems[t % 3], 16)

            if with_compute:
                issue_patch(0)
                issue_patch(1)
                issue_patch(2)

            if with_tail:
                sync.wait_ge(z_sem, 1)
                for g in range(4):
                    sync.dma_start(
                        out=AP(canv, g * OUT_FLAT, [[800, 128], [1, 800]]),
                        in_=zero_t[:, :],
                    ).then_inc(zc_sem, 16)

            def issue_fills(t, ils=range(TILE_I), px_half=None):
                # scatter Pbuf[:, t%2] (tile t's 3 i-rows) into parity canvases.
                # i = 3t+il; a = i%2; canvas row gx = 4i+px; cols gy = 4j+py.
                # px_half: None = all 8 px rows; 0/1 = lower/upper 4 (k halves)
                if px_half is None:
                    pxo, npx, ko = 0, F, slice(None)
                elif px_half == 0:
                    pxo, npx, ko = 0, F // 2, slice(0, K // 2)
                else:
                    pxo, npx, ko = F // 2, F // 2, slice(K // 2, K)
                for il in ils:
                    i = TILE_I * t + il
                    a = i % 2
                    for b in range(2):
                        nj = (W_OUT - b + 1) // 2
                        p0 = il * W_OUT + (0 if b == 0 else (W_OUT + 1) // 2)
                        g = 2 * a + b
                        sync.dma_start(
                            out=AP(
                                canv,
                                g * OUT_FLAT + (4 * i + pxo) * xs_r + 4 * b * xs_c,
                                [[8 * xs_c, nj], [xs_r, npx], [1, F * C_IN]],
                            ),
                            in_=Pbuf[p0 : p0 + nj, t % 2, ko],
                        ).then_inc(fill_sems[t % 2], 16)

            if with_compute:
                for t in range(N_TILES):
                    # prefetch patch(t+3): overwrites buf t%3, last read by
                    # mult(t) (vt hits 2t+1 when mult(t) completes)
                    if t + 3 < N_TILES:
                        sync.wait_ge(vt_sem, 2 * t + 1)
                        sync.wait_ge(pe_sem, t)
                        issue_patch(t + 3)
                    sync.wait_ge(vr_sem, t + 1)
                    if with_tail:
                        if t == 0:
                            sync.wait_ge(zc_sem, 16 * 4)
                        # last tile: il=2 fills go out on the scalar queue
                        issue_fills(t, ils=(0, 1) if t == N_TILES - 1 else range(TILE_I))
                        if t == N_TILES - 1:
                            # low-row readback of canvases 0:2 (final after
                            # fills(11)/fills(10); tile 12 only writes gx>=144,
                            # disjoint from rows <140 read here)
                            sync.dma_start(
                                out=bigrb[0:112, 0:2, :],
                                in_=AP(canv, 0, [[800, 112], [OUT_FLAT, 2], [1, 800]]),
                            ).then_inc(rbb_sem0, 16)

            if with_tail:
                if with_compute:
                    sync.wait_ge(
                        fill_sems[0],
                        16 * (FILLS_PER_TILE * n_fill(N_TILES - 3) + 4),
                    )
                    sync.wait_ge(fill_sems[1], 16 * FILLS_PER_TILE * n_fill(N_TILES - 2))
                    sync.wait_ge(fsc_sem, 16 * 2)
                else:
                    sync.wait_ge(zc_sem, 16 * 4)
                # high rows (gx>=140): all 4 canvases, small
                sync.dma_start(
                    out=bigrb[112:128, :, :],
                    in_=AP(canv, 112 * 800, [[800, 16], [OUT_FLAT, 4], [1, 800]]),
                ).then_inc(rbb_sem0, 16)

                # after DVE summed + cast bf16: push to ar_in
                sync.wait_ge(va_sem, 1)
                sync.dma_start(
                    out=AP(ar_in, 0, [[800, 128], [1, 800]]),
                    in_=acc_bf[:, :],
                ).then_inc(ar_sem, 16)

                sync.wait_ge(cc_sem, 1)
                sync.dma_start(
                    out=AP(out_ext, 0, [[100, 128], [1, 100]]),
                    in_=AP(rs_out, 0, [[100, 128], [1, 100]]),
                ).then_inc(ar_sem, 16)
                sync.wait_ge(ar_sem, 32)

        # ---------------- DVE: main compute ----------------
        @block.vector
        def _(vector: bass.BassVectorEngine):
            vector.memset(zero_t[:, :], 0.0).then_inc(z_sem, 1)

            if with_compute:
                vector.wait_ge(zw_sem, 16)
                vector.wait_ge(zwb_sem, 16)  # tile 0 uses the full w

                CL2 = 2  # prod split: DVE computes c<CL2, Pool computes c>=CL2
                for t in range(N_TILES + 1):
                    if t < N_TILES:
                        cl = C_SH if t == 0 else CL2  # tile 0 fully on DVE
                        vector.wait_ge(patch_sems[t % 3], 16 * (t // 3 + 1))
                        if t >= 2:
                            # prod[t%2] WAR: eq(t-2) must be done reading it
                            vector.wait_ge(ve_sem, t - 1)
                        vector.tensor_tensor(
                            out=prod[:, t % 2, :cl, :],
                            in0=patch_sb[:, t % 3, :]
                            .unsqueeze(1)
                            .to_broadcast([NP, cl, K]),
                            in1=w_rep[:, :cl, :],
                            op=Alu.mult,
                        ).then_inc(vt_sem, 1)
                        vector.wait_ge(vt_sem, 2 * t + 1)
                        if t >= 1:
                            # Pool's prod half must be in before the max
                            vector.wait_ge(pe_sem, t)
                        vector.tensor_reduce(
                            out=mvals[:, :],
                            in_=prod[:, t % 2, :, :],
                            axis=Axis.X,
                            op=Alu.max,
                        ).then_inc(vt_sem, 1)
                        vector.wait_ge(vt_sem, 2 * t + 2)
                        if t >= 2:
                            # onehot[t%2] overwrite: Pool mult(t-2) done
                            vector.wait_ge(p_sem, t - 1)
                        vector.tensor_tensor(
                            out=onehot[:, t % 2, :, :].transpose([0, 2, 1]),
                            in0=prod[:, t % 2, :, :],
                            in1=mvals[:, :].unsqueeze(2).to_broadcast([NP, C_SH, K]),
                            op=Alu.is_equal,
                        ).then_inc(ve_sem, 1)
                    if t >= 1:
                        tr = t - 1
                        vector.wait_ge(p_sem, tr + 1)
                        if with_tail and tr >= 2:
                            # Pbuf[tr%2] reuse: fills(tr-2) must have drained it
                            vector.wait_ge(
                                fill_sems[tr % 2], 16 * FILLS_PER_TILE * n_fill(tr - 2)
                            )
                        with nc.allow_low_precision(
                            "bf16 relevance sums stay well inside the 2e-2 gate"
                        ):
                            vector.tensor_reduce(
                                out=Pbuf[:, tr % 2, :],
                                in_=onehot[:, tr % 2, :, :],
                                axis=Axis.X,
                                op=Alu.add,
                            ).then_inc(vr_sem, 1)

            if with_tail:
                nvt = 2 * N_TILES if with_compute else 0
                vector.wait_ge(rbb_sem0, 32)
                vector.tensor_tensor(
                    out=bigrb[:, 0, :], in0=bigrb[:, 0, :], in1=bigrb[:, 1, :],
                    op=Alu.add,
                ).then_inc(vt_sem, 1)
                vector.wait_ge(rbb_sem1, 16)
                vector.tensor_tensor(
                    out=bigrb[:, 2, :], in0=bigrb[:, 2, :], in1=bigrb[:, 3, :],
                    op=Alu.add,
                ).then_inc(vt_sem, 1)
                vector.wait_ge(vt_sem, nvt + 2)
                # final add writes f32 directly (dtype converts on write)
                vector.tensor_tensor(
                    out=acc_bf[:, :], in0=bigrb[:, 0, :], in1=bigrb[:, 2, :],
                    op=Alu.add,
                ).then_inc(va_sem, 1)

        # ---------------- Pool: rel multiply + collective ----------------
        @block.gpsimd
        def _(gpsimd: bass.BassGpSimd):
            if with_compute:
                CL2 = 2
                # fetch own w half on the Pool DMA queue, parallel to scalar's
                gpsimd.dma_start(
                    out=w_rep[:, 2:, :],
                    in_=AP(w_ext, 2 * K, [[0, NP], [K, C_SH - 2], [1, K]]),
                ).then_inc(zwb_sem, 16)
                gpsimd.wait_ge(zwb_sem, 16)
                gpsimd.wait_ge(rb_sem, 16)  # rel_bf loaded

                def rel_mult(t):
                    gpsimd.wait_ge(ve_sem, t + 1)
                    gpsimd.tensor_tensor(
                        out=onehot[:, t % 2, :, :],
                        in0=onehot[:, t % 2, :, :],
                        in1=rel_bf[:, t, :].unsqueeze(1).to_broadcast([NP, K, C_SH]),
                        op=Alu.mult,
                    ).then_inc(p_sem, 1)

                for t in range(1, N_TILES):
                    # upper prod half: prod[:, t%2, CL2:, :] = patch * w
                    gpsimd.wait_ge(patch_sems[t % 3], 16 * (t // 3 + 1))
                    if t >= 2:
                        # prod[t%2] WAR: eq(t-2) must be done reading it
                        gpsimd.wait_ge(ve_sem, t - 1)
                    gpsimd.tensor_tensor(
                        out=prod[:, t % 2, CL2:, :],
                        in0=patch_sb[:, t % 3, :]
                        .unsqueeze(1)
                        .to_broadcast([NP, C_SH - CL2, K]),
                        in1=w_rep[:, CL2:, :],
                        op=Alu.mult,
                    ).then_inc(pe_sem, 1)
                    rel_mult(t - 1)
                rel_mult(N_TILES - 1)

            if with_tail:
                gpsimd.wait_ge(ar_sem, 16)
                gpsimd.collective_compute(
                    "ReduceScatter",
                    Alu.add,
                    replica_groups=[list(range(N_CORES))],
                    ins=[ar_in[:]],
                    outs=[rs_out[:]],
                ).then_inc(cc_sem, 1)

    return nc


_NC = None


def _get_nc():
    global _NC
    if _NC is None:
        _NC = _build_nc()
    return _NC


LAST_RESULT = None


def kernel(inputs, layer_output, layer_weights, stride=4, filter_size=8, **_kw):
    assert int(stride) == S and int(filter_size) == F
    rel = np.asarray(inputs, dtype=np.float32)[0]          # [39,39,64]
    x = np.ascontiguousarray(np.asarray(layer_output, dtype=np.float32)[0])
    w = np.asarray(layer_weights, dtype=np.float32)        # [8,8,4,64]

    # host-side im2col in the kernel's (il*39+jp, t, k) layout, j parity-permuted
    j_order = list(range(0, W_OUT, 2)) + list(range(1, W_OUT, 2))
    idx_r = (S * np.arange(H_OUT))[:, None] + np.arange(F)[None, :]
    idx_c = (S * np.asarray(j_order))[:, None] + np.arange(F)[None, :]
    pat = x[idx_r][:, :, idx_c, :]                    # [i, px, jp, py, pc]
    pat = pat.transpose(0, 2, 1, 3, 4).reshape(H_OUT, W_OUT, K)
    patches = np.ascontiguousarray(
        pat.reshape(N_TILES, TILE_I, W_OUT, K).reshape(N_TILES, NP, K)
    )

    from concourse.bass_utils import run_bass_kernel_spmd

    nc = _get_nc()
    in_maps = []
    for r in range(N_CORES):
        cs = slice(C_SH * r, C_SH * (r + 1))
        w_t = np.ascontiguousarray(
            w[:, :, :, cs].transpose(3, 0, 1, 2).reshape(C_SH, K)
        )
        j_order = list(range(0, W_OUT, 2)) + list(range(1, W_OUT, 2))
        import ml_dtypes

        rel_r = np.ascontiguousarray(
            rel[:, j_order, :][:, :, cs]
            .reshape(N_TILES, TILE_I, W_OUT, C_SH)
            .transpose(1, 2, 0, 3)
            .reshape(NP, N_TILES, C_SH)
            .astype(ml_dtypes.bfloat16)
        )
        in_maps.append({"patches": patches, "w": w_t, "rel": rel_r})

    import os

    trace = bool(int(os.environ.get("KERNEL_TRACE", "0")))
    res = run_bass_kernel_spmd(nc, in_maps, list(range(N_CORES)), trace=trace)
    global LAST_RESULT
    LAST_RESULT = res
    slices = [np.asarray(res.results[r]["out"]) for r in range(N_CORES)]
    out = np.concatenate(slices, axis=0).reshape(1, H_IN, W_IN, C_IN)
    return out.astype(np.float32)

